# revision 1
# baseline (speedup 1.0000x reference)
"""Additive (Bahdanau) attention on 8 TRN2 NeuronCores.

Problem shapes: B=8, T=128, S=512, A=256 (f32).
  q = queries @ W_q.T + b_q                  [B,T,A]
  k = keys @ W_k.T + b_k                     [B,S,A]
  scores[b,t,s] = sum_a v_a[a]*tanh(q[b,t,a]+k[b,s,a]) + b_a
  out = softmax_s(scores) @ values           [B,T,A]

Sharding: pure data-parallel over B — core i computes batch i. Weights
replicated. No collectives.

Per-core kernel strategy:
  - Transpose keys/queries/W_q/W_k on-chip (PE transposes) so the
    contraction dim `a` lands on partitions.
  - kp[c,s] (c=a-out-channel on partitions, s free) and qp[c,t] via PE.
  - Hot loop over t: DVE tensor_scalar_add broadcasts qp[:,t] over
    kp [128,512] (two a-halves) writing bf16; ACT does one big
    batched tanh; PE contracts over `a` with lhsT=tanh tile [a,s-block],
    rhs=v column (N=1) accumulating scores^T [s,t] into a single PSUM
    bank.
  - softmax: shift-invariance drops b_a and the max-subtraction
    entirely (|scores| <~ 13 so exp is safe in f32). One ACT exp
    PSUM->SBUF. Sums over s come from the out-matmul by appending a
    ones column to values: out_psum[t, 0:256]=unnormalized out,
    out_psum[t,256]=sum. DVE reciprocal + tensor_scalar_mul, DMA out.
"""

import numpy as np

import concourse.bacc as bacc
import concourse.mybir as mybir
import concourse.tile as tile
from concourse.bass_utils import run_bass_kernel_spmd
from concourse.masks import make_identity

F32 = mybir.dt.float32
BF16 = mybir.dt.bfloat16
AF = mybir.ActivationFunctionType

B, T, S, A = 8, 128, 512, 256
AH = A // 128  # a-halves (2)
SB = S // 128  # s-blocks (4)
CH = A // 128  # c-halves of the projected dim (2)
# t-batch sizes per tanh instruction: small first batches cut the
# DVE->ACT startup latency; small last batches cut the PE tail.
BATCHES = [3, 3, 4, 6, 8, 10, 13, 14, 14, 14, 14, 14, 8, 2, 1]
assert sum(BATCHES) == T

N_CORES = 8


def build_nc(batches=None):
    if batches is None:
        batches = BATCHES
    nc = bacc.Bacc("TRN2", target_bir_lowering=False, debug=False,
                   num_devices=N_CORES)

    queries = nc.dram_tensor("queries", [T, A], F32, kind="ExternalInput")
    keys = nc.dram_tensor("keys", [S, A], F32, kind="ExternalInput")
    values = nc.dram_tensor("values", [S, A], F32, kind="ExternalInput")
    W_q = nc.dram_tensor("W_q", [A, A], F32, kind="ExternalInput")
    b_q = nc.dram_tensor("b_q", [A], F32, kind="ExternalInput")
    W_k = nc.dram_tensor("W_k", [A, A], F32, kind="ExternalInput")
    b_k = nc.dram_tensor("b_k", [A], F32, kind="ExternalInput")
    v_a = nc.dram_tensor("v_a", [1, A], F32, kind="ExternalInput")
    out_d = nc.dram_tensor("out", [T, A], F32, kind="ExternalOutput")

    with tile.TileContext(nc) as tc:
        with (
            tc.tile_pool(name="persist", bufs=1) as pp,
            tc.tile_pool(name="stage", bufs=3) as sp,
            tc.tile_pool(name="u", bufs=2) as up,
            tc.tile_pool(name="ut", bufs=3) as utp,
            tc.tile_pool(name="psum_t", bufs=4, space="PSUM") as ptp,
            tc.tile_pool(name="psum_p", bufs=2, space="PSUM") as ppp,
            tc.tile_pool(name="psum_s", bufs=1, space="PSUM") as psp,
        ):
            ident = pp.tile([128, 128], BF16, tag="ident")
            make_identity(nc, ident[:])

            # ---- persistent SBUF layouts (partition dim = a or c) ----
            kT = pp.tile([128, AH, S], BF16, tag="kT")       # keys^T
            qT = pp.tile([128, AH, T], BF16, tag="qT")       # queries^T
            WqT = pp.tile([128, AH, A], BF16, tag="WqT")     # W_q^T
            WkT = pp.tile([128, AH, A], BF16, tag="WkT")     # W_k^T
            kp = pp.tile([128, CH, S], BF16, tag="kp")       # k-proj [c,s]
            qp = pp.tile([128, CH, T], F32, tag="qp")        # q-proj [c,t]
            bqc = pp.tile([128, CH], F32, tag="bqc")
            bkc = pp.tile([128, CH], F32, tag="bkc")
            vf = pp.tile([128, AH], F32, tag="vf")
            vb = pp.tile([128, AH], BF16, tag="vb")          # v_a bf16 cols
            vaug = pp.tile([128, SB, A + 1], BF16, tag="vaug")  # [values|1]
            vnat = pp.tile([128, SB, A], F32, tag="vnat")
            wT = pp.tile([128, SB, T], BF16, tag="wT")       # exp(scores)^T
            out_sb = pp.tile([128, A], F32, tag="out_sb")

            # ---- load + transpose inputs ----
            def load_transposed(dram, rows, dst, eng):
                # dram [rows, A] -> dst [128, AH, rows] (dst[p, h, r] =
                # dram[r, h*128+p]).  f32 DMA in, DVE cast to bf16, then
                # PE transpose + DVE copy out of PSUM.
                for rb in range(rows // 128):
                    nat = sp.tile([128, A], F32, tag="nat")
                    eng.dma_start(nat[:], dram[rb * 128:(rb + 1) * 128, :])
                    natb = sp.tile([128, A], BF16, tag="natb")
                    nc.vector.tensor_copy(out=natb[:], in_=nat[:])
                    for h in range(AH):
                        ps = ptp.tile([128, 128], BF16, tag="tps")
                        nc.tensor.transpose(
                            ps[:], natb[:, h * 128:(h + 1) * 128], ident[:])
                        nc.vector.tensor_copy(
                            out=dst[:, h, rb * 128:(rb + 1) * 128], in_=ps[:])

            # bias columns / v_a first (tiny, gpsimd)
            nc.gpsimd.dma_start(bqc[:], b_q[:].rearrange("(h p) -> p h", p=128))
            nc.gpsimd.dma_start(bkc[:], b_k[:].rearrange("(h p) -> p h", p=128))
            nc.gpsimd.dma_start(vf[:], v_a[0, :].rearrange("(h p) -> p h", p=128))
            nc.vector.tensor_copy(out=vb[:], in_=vf[:])

            # q-side first (small) so qp is ready early
            load_transposed(W_q, A, WqT, nc.scalar)
            load_transposed(queries, T, qT, nc.scalar)
            for ch in range(CH):
                pq = ptp.tile([128, T], F32, tag="tps")
                for h in range(AH):
                    nc.tensor.matmul(
                        pq[:], WqT[:, h, ch * 128:(ch + 1) * 128], qT[:, h, :],
                        start=(h == 0), stop=(h == AH - 1))
                nc.vector.tensor_scalar_add(
                    out=qp[:, ch, :], in0=pq[:], scalar1=bqc[:, ch:ch + 1])

            # k-side: per keys-chunk, transpose then immediately project
            # that s-block so the fused batch-0 tanh can start after the
            # first chunk instead of after the whole of keys.
            load_transposed(W_k, A, WkT, nc.sync)
            pk0 = ppp.tile([128, S], F32, tag="pk")
            pk1 = ppp.tile([128, S], F32, tag="pk")
            pks = [pk0, pk1]
            for sb in range(SB):
                nat = sp.tile([128, A], F32, tag="nat")
                nc.sync.dma_start(nat[:], keys[sb * 128:(sb + 1) * 128, :])
                natb = sp.tile([128, A], BF16, tag="natb")
                nc.vector.tensor_copy(out=natb[:], in_=nat[:])
                for h in range(AH):
                    ps = ptp.tile([128, 128], BF16, tag="tps")
                    nc.tensor.transpose(
                        ps[:], natb[:, h * 128:(h + 1) * 128], ident[:])
                    nc.vector.tensor_copy(
                        out=kT[:, h, sb * 128:(sb + 1) * 128], in_=ps[:])
                for ch in range(CH):
                    for h in range(AH):
                        nc.tensor.matmul(
                            pks[ch][:, sb * 128:(sb + 1) * 128],
                            WkT[:, h, ch * 128:(ch + 1) * 128],
                            kT[:, h, sb * 128:(sb + 1) * 128],
                            start=(h == 0), stop=(h == AH - 1))
                    nc.vector.tensor_scalar_add(
                        out=kp[:, ch, sb * 128:(sb + 1) * 128],
                        in0=pks[ch][:, sb * 128:(sb + 1) * 128],
                        scalar1=bkc[:, ch:ch + 1])

            # values + ones column (needed only at the end)
            for sb in range(SB):
                nc.gpsimd.dma_start(vnat[:, sb, :],
                                    values[sb * 128:(sb + 1) * 128, :])
                nc.vector.tensor_copy(out=vaug[:, sb, :A], in_=vnat[:, sb, :])
            nc.vector.memset(vaug[:, :, A:A + 1], 1.0)

            # scores^T accumulator: [s(part), sb, t] — one PSUM bank
            scT = psp.tile([128, SB, T], F32, tag="scT")

            # ---- hot loop ----
            t0 = 0
            for bi, tb in enumerate(batches):
                ut = utp.tile([128, AH, tb * S], BF16, tag="ut")
                if bi == 0:
                    # fused add+tanh on ACT (per-partition bias = qp col),
                    # per s-halfblock: starts as soon as kp slices land,
                    # without waiting on DVE adds or the full projection.
                    for sbp in range(SB // 2):
                        for i in range(tb):
                            t = t0 + i
                            for h in range(AH):
                                nc.scalar.activation(
                                    ut[:, h,
                                       i * S + sbp * 256:i * S + (sbp + 1) * 256],
                                    kp[:, h, sbp * 256:(sbp + 1) * 256],
                                    AF.Tanh, bias=qp[:, h, t:t + 1])
                else:
                    u = up.tile([128, AH, tb * S], BF16, tag="u")
                    for i in range(tb):
                        t = t0 + i
                        for h in range(AH):
                            if bi == 1:
                                # split per s-block: consumes kp slices as
                                # the pipelined k-projection produces them
                                for sb in range(SB):
                                    nc.vector.tensor_scalar_add(
                                        out=u[:, h, i * S + sb * 128:
                                              i * S + (sb + 1) * 128],
                                        in0=kp[:, h, sb * 128:(sb + 1) * 128],
                                        scalar1=qp[:, h, t:t + 1])
                            else:
                                nc.vector.tensor_scalar_add(
                                    out=u[:, h, i * S:(i + 1) * S],
                                    in0=kp[:, h, :],
                                    scalar1=qp[:, h, t:t + 1])
                    nc.scalar.activation(ut[:], u[:], AF.Tanh)
                for i in range(tb):
                    t = t0 + i
                    for sb in range(SB):
                        for h in range(AH):
                            nc.tensor.matmul(
                                scT[:, sb, t:t + 1],
                                ut[:, h, i * S + sb * 128:i * S + (sb + 1) * 128],
                                vb[:, h:h + 1],
                                start=(h == 0), stop=(h == AH - 1))
                t0 += tb

            # ---- softmax + output ----
            nc.scalar.activation(wT[:], scT[:], AF.Exp)
            po = ppp.tile([128, A + 1], F32, tag="pk")
            for sb in range(SB):
                nc.tensor.matmul(po[:], wT[:, sb, :], vaug[:, sb, :],
                                 start=(sb == 0), stop=(sb == SB - 1))
            rs = pp.tile([128, 1], F32, tag="rs")
            nc.vector.reciprocal(out=rs[:], in_=po[:, A:A + 1])
            nc.vector.tensor_scalar_mul(out=out_sb[:], in0=po[:, :A],
                                        scalar1=rs[:])
            nc.sync.dma_start(out_d[:, :], out_sb[:])

    nc.compile()
    return nc


_NC = None


def _get_nc():
    global _NC
    if _NC is None:
        _NC = build_nc()
    return _NC


def make_in_maps(queries, keys, values, W_q, b_q, W_k, b_k, v_a):
    f = lambda x: np.ascontiguousarray(x, dtype=np.float32)
    return [
        {
            "queries": f(queries[i]),
            "keys": f(keys[i]),
            "values": f(values[i]),
            "W_q": f(W_q),
            "b_q": f(b_q),
            "W_k": f(W_k),
            "b_k": f(b_k),
            "v_a": f(v_a),
        }
        for i in range(N_CORES)
    ]


def run(nc, in_maps, **kw):
    res = run_bass_kernel_spmd(nc, in_maps, core_ids=list(range(N_CORES)), **kw)
    out = np.stack([res.results[i]["out"] for i in range(N_CORES)], axis=0)
    return out, res


def kernel(queries, keys, values, W_q, b_q, W_k, b_k, v_a, b_a=None, **_):
    # b_a shifts all scores equally -> softmax-invariant -> unused.
    nc = _get_nc()
    in_maps = make_in_maps(queries, keys, values, W_q, b_q, W_k, b_k, v_a)
    # The kernel is deterministic, but the shared device has shown rare
    # transient execution corruption: require two consecutive runs to
    # agree bit-exactly before returning.
    prev = None
    for _ in range(5):
        out, _res = run(nc, in_maps)
        if prev is not None and np.array_equal(out, prev):
            break
        prev = out
    return out.astype(np.float32)



# revision 4
# speedup vs baseline: 1.0007x; 1.0007x over previous
"""Additive (Bahdanau) attention on 8 TRN2 NeuronCores.

Problem shapes: B=8, T=128, S=512, A=256 (f32).
  q = queries @ W_q.T + b_q                  [B,T,A]
  k = keys @ W_k.T + b_k                     [B,S,A]
  scores[b,t,s] = sum_a v_a[a]*tanh(q[b,t,a]+k[b,s,a]) + b_a
  out = softmax_s(scores) @ values           [B,T,A]

Sharding: pure data-parallel over B — core i computes batch i. Weights
replicated. No collectives.

Per-core kernel strategy (ACT-engine bound: the 16.8M-element tanh is a
hard ~109us floor at 1 elem/lane/cycle; everything else hides under it):
  - Prologue on 3 parallel DMA queues (sync: W_k+keys, scalar:
    W_q+queries, gpsimd: biases/v_a/values) into dedicated tiles; PE
    transposes run on the f32 data directly and the bf16 cast rides the
    PSUM->SBUF copy-out, so the first tanh starts ~10.5us in instead of
    19us.
  - Hot loop over t-batches: DVE tensor_scalar_add broadcasts qp[:,t]
    over kp [128,512] writing bf16; ACT does one big batched tanh per
    batch; PE contracts over `a` with lhsT=tanh tile [a,s-block],
    rhs=v column (N=1) accumulating scores^T [s,t] into one PSUM bank.
    Batch sizes ramp (2 fused, then 4,5,...) so ACT never outruns the
    DVE adds.
  - softmax: shift-invariance drops b_a and max-subtraction (|scores|
    <~ 13 so exp is safe in f32). Sums over s come from the out-matmul
    via a ones column appended to values. The exp/out-matmul/normalize/
    store epilogue runs in two t-halves: the first half mid-loop under
    the tanh shadow, only the second-half tail (~2us) is exposed.
"""

import numpy as np

import concourse.bacc as bacc
import concourse.mybir as mybir
import concourse.tile as tile
from concourse.bass_utils import run_bass_kernel_spmd
from concourse.masks import make_identity

F32 = mybir.dt.float32
BF16 = mybir.dt.bfloat16
AF = mybir.ActivationFunctionType

B, T, S, A = 8, 128, 512, 256
AH = A // 128  # a-halves (2)
SB = S // 128  # s-blocks (4)
CH = A // 128  # c-halves of the projected dim (2)
# t-batch sizes per tanh instruction: batch 0 is ACT-fused (bias=qp col,
# no DVE dependency); later batches ramp so the DVE adds stay ahead of
# the batched ACT tanh. Small last batch cuts the PE score-matmul tail.
BATCHES = [2, 4, 6, 6, 8, 10, 14, 14, 14, 14, 14, 12, 8, 2]
assert sum(BATCHES) == T
SPLIT_BI = 7  # emit first-half epilogue after this batch index (cum t=64,
# must be a multiple of 32: PE matmul base partition constraint)

N_CORES = 8


def build_nc(batches=None, split_bi=None):
    if batches is None:
        batches = BATCHES
    if split_bi is None:
        split_bi = SPLIT_BI
    split_t = sum(batches[:split_bi + 1])
    nc = bacc.Bacc("TRN2", target_bir_lowering=False, debug=False,
                   num_devices=N_CORES)

    queries = nc.dram_tensor("queries", [T, A], F32, kind="ExternalInput")
    keys = nc.dram_tensor("keys", [S, A], F32, kind="ExternalInput")
    values = nc.dram_tensor("values", [S, A], F32, kind="ExternalInput")
    W_q = nc.dram_tensor("W_q", [A, A], F32, kind="ExternalInput")
    b_q = nc.dram_tensor("b_q", [A], F32, kind="ExternalInput")
    W_k = nc.dram_tensor("W_k", [A, A], F32, kind="ExternalInput")
    b_k = nc.dram_tensor("b_k", [A], F32, kind="ExternalInput")
    v_a = nc.dram_tensor("v_a", [1, A], F32, kind="ExternalInput")
    out_d = nc.dram_tensor("out", [T, A], F32, kind="ExternalOutput")

    with tile.TileContext(nc) as tc:
        with (
            tc.tile_pool(name="persist", bufs=1) as pp,
            tc.tile_pool(name="u", bufs=3) as up,
            tc.tile_pool(name="ut", bufs=3) as utp,
            tc.tile_pool(name="psum_t", bufs=3, space="PSUM") as ptp,
            tc.tile_pool(name="psum_p", bufs=2, space="PSUM") as ppp,
            tc.tile_pool(name="psum_s", bufs=1, space="PSUM") as psp,
        ):
            ident = pp.tile([128, 128], F32, tag="ident")

            # ---- persistent SBUF layouts (partition dim = a or c) ----
            natWk = pp.tile([128, 2, A], F32, tag="natWk")
            natWq = pp.tile([128, 2, A], F32, tag="natWq")
            natQ = pp.tile([128, A], F32, tag="natQ")
            natK = pp.tile([128, SB, A], F32, tag="natK")
            kT = pp.tile([128, AH, S], BF16, tag="kT")       # keys^T
            qT = pp.tile([128, AH, T], BF16, tag="qT")       # queries^T
            WqT = pp.tile([128, AH, A], BF16, tag="WqT")     # W_q^T
            WkT = pp.tile([128, AH, A], BF16, tag="WkT")     # W_k^T
            kp = pp.tile([128, CH, S], BF16, tag="kp")       # k-proj [c,s]
            qp = pp.tile([128, CH, T], F32, tag="qp")        # q-proj [c,t]
            bqc = pp.tile([128, CH], F32, tag="bqc")
            bkc = pp.tile([128, CH], F32, tag="bkc")
            vf = pp.tile([128, AH], F32, tag="vf")
            vb = pp.tile([128, AH], BF16, tag="vb")          # v_a bf16 cols
            vaug = pp.tile([128, SB, A + 1], BF16, tag="vaug")  # [values|1]
            vnat = pp.tile([128, SB, A], F32, tag="vnat")
            wT = pp.tile([128, SB, T], BF16, tag="wT")       # exp(scores)^T
            out_sb = pp.tile([128, A], F32, tag="out_sb")
            rs = pp.tile([128, 1], F32, tag="rs")

            # ---- phase A: all input DMAs up front, 3 parallel queues ----
            make_identity(nc, ident[:])  # gpsimd, ~0.4us, gates transposes
            nc.gpsimd.dma_start(bqc[:], b_q[:].rearrange("(h p) -> p h", p=128))
            nc.gpsimd.dma_start(bkc[:], b_k[:].rearrange("(h p) -> p h", p=128))
            nc.gpsimd.dma_start(vf[:], v_a[0, :].rearrange("(h p) -> p h", p=128))
            nc.sync.dma_start(natWk[:], W_k[:].rearrange("(r p) a -> p r a", p=128))
            for sb in range(SB):
                nc.sync.dma_start(natK[:, sb, :],
                                  keys[sb * 128:(sb + 1) * 128, :])
            nc.scalar.dma_start(natWq[:], W_q[:].rearrange("(r p) a -> p r a", p=128))
            nc.scalar.dma_start(natQ[:], queries[:, :])
            for sb in range(SB):
                nc.gpsimd.dma_start(vnat[:, sb, :],
                                    values[sb * 128:(sb + 1) * 128, :])
            nc.vector.tensor_copy(out=vb[:], in_=vf[:])

            # ---- phase B: transposes (f32 in, bf16 out via copy) ----
            def transpose_to(nat_slice, dst_slice):
                ps = ptp.tile([128, 128], F32, tag="tps")
                nc.tensor.transpose(ps[:], nat_slice, ident[:])
                nc.vector.tensor_copy(out=dst_slice, in_=ps[:])

            for h in range(AH):
                for rb in range(2):
                    transpose_to(natWk[:, rb, h * 128:(h + 1) * 128],
                                 WkT[:, h, rb * 128:(rb + 1) * 128])
            for h in range(AH):
                transpose_to(natK[:, 0, h * 128:(h + 1) * 128],
                             kT[:, h, 0:128])
            for h in range(AH):
                for rb in range(2):
                    transpose_to(natWq[:, rb, h * 128:(h + 1) * 128],
                                 WqT[:, h, rb * 128:(rb + 1) * 128])
            for h in range(AH):
                transpose_to(natQ[:, h * 128:(h + 1) * 128], qT[:, h, :])

            # k-proj s-block 0, then q-proj, then remaining k s-blocks
            pk0 = ppp.tile([128, S], F32, tag="pk")
            pk1 = ppp.tile([128, S], F32, tag="pk")
            pks = [pk0, pk1]

            def kproj(sb):
                for ch in range(CH):
                    for h in range(AH):
                        nc.tensor.matmul(
                            pks[ch][:, sb * 128:(sb + 1) * 128],
                            WkT[:, h, ch * 128:(ch + 1) * 128],
                            kT[:, h, sb * 128:(sb + 1) * 128],
                            start=(h == 0), stop=(h == AH - 1))
                    nc.vector.tensor_scalar_add(
                        out=kp[:, ch, sb * 128:(sb + 1) * 128],
                        in0=pks[ch][:, sb * 128:(sb + 1) * 128],
                        scalar1=bkc[:, ch:ch + 1])

            kproj(0)
            for ch in range(CH):
                pq = ptp.tile([128, T], F32, tag="tps")
                for h in range(AH):
                    nc.tensor.matmul(
                        pq[:], WqT[:, h, ch * 128:(ch + 1) * 128], qT[:, h, :],
                        start=(h == 0), stop=(h == AH - 1))
                nc.vector.tensor_scalar_add(
                    out=qp[:, ch, :], in0=pq[:], scalar1=bqc[:, ch:ch + 1])
            for sb in range(1, SB):
                for h in range(AH):
                    transpose_to(natK[:, sb, h * 128:(h + 1) * 128],
                                 kT[:, h, sb * 128:(sb + 1) * 128])
                kproj(sb)

            # values + ones column (gpsimd; needed only by the epilogue)
            for sb in range(SB):
                nc.gpsimd.tensor_copy(out=vaug[:, sb, :A], in_=vnat[:, sb, :])
            nc.gpsimd.memset(vaug[:, :, A:A + 1], 1.0)

            # scores^T accumulator: [s(part), sb, t] — one PSUM bank
            scT = psp.tile([128, SB, T], F32, tag="scT")
            po = ppp.tile([128, A + 1], F32, tag="pk")

            def epilogue(lo, hi):
                # exp -> out-matmul (with sums via ones col) -> normalize
                nc.scalar.activation(wT[:, :, lo:hi], scT[:, :, lo:hi], AF.Exp)
                for sb in range(SB):
                    nc.tensor.matmul(po[lo:hi, :], wT[:, sb, lo:hi],
                                     vaug[:, sb, :],
                                     start=(sb == 0), stop=(sb == SB - 1))
                nc.vector.reciprocal(out=rs[lo:hi], in_=po[lo:hi, A:A + 1])
                nc.vector.tensor_scalar_mul(out=out_sb[lo:hi, :],
                                            in0=po[lo:hi, :A],
                                            scalar1=rs[lo:hi])
                nc.sync.dma_start(out_d[lo:hi, :], out_sb[lo:hi, :])

            # ---- phase C: hot loop ----
            t0 = 0
            for bi, tb in enumerate(batches):
                ut = utp.tile([128, AH, tb * S], BF16, tag="ut")
                if bi == 0:
                    # fused add+tanh on ACT (per-partition bias = qp col),
                    # per s-halfblock: starts as soon as kp/qp land,
                    # without waiting on any DVE adds.
                    for sbp in range(SB // 2):
                        for i in range(tb):
                            t = t0 + i
                            for h in range(AH):
                                nc.scalar.activation(
                                    ut[:, h,
                                       i * S + sbp * 256:i * S + (sbp + 1) * 256],
                                    kp[:, h, sbp * 256:(sbp + 1) * 256],
                                    AF.Tanh, bias=qp[:, h, t:t + 1])
                else:
                    u = up.tile([128, AH, tb * S], BF16, tag="u")
                    for i in range(tb):
                        t = t0 + i
                        for h in range(AH):
                            nc.vector.tensor_scalar_add(
                                out=u[:, h, i * S:(i + 1) * S],
                                in0=kp[:, h, :],
                                scalar1=qp[:, h, t:t + 1])
                    nc.scalar.activation(ut[:], u[:], AF.Tanh)
                for i in range(tb):
                    t = t0 + i
                    for sb in range(SB):
                        for h in range(AH):
                            nc.tensor.matmul(
                                scT[:, sb, t:t + 1],
                                ut[:, h, i * S + sb * 128:i * S + (sb + 1) * 128],
                                vb[:, h:h + 1],
                                start=(h == 0), stop=(h == AH - 1))
                t0 += tb
                if bi == split_bi:
                    epilogue(0, split_t)

            # ---- phase D: second-half epilogue ----
            epilogue(split_t, T)

    nc.compile()
    return nc


_NC = None


def _get_nc():
    global _NC
    if _NC is None:
        _NC = build_nc()
    return _NC


def make_in_maps(queries, keys, values, W_q, b_q, W_k, b_k, v_a):
    f = lambda x: np.ascontiguousarray(x, dtype=np.float32)
    return [
        {
            "queries": f(queries[i]),
            "keys": f(keys[i]),
            "values": f(values[i]),
            "W_q": f(W_q),
            "b_q": f(b_q),
            "W_k": f(W_k),
            "b_k": f(b_k),
            "v_a": f(v_a),
        }
        for i in range(N_CORES)
    ]


def run(nc, in_maps, **kw):
    res = run_bass_kernel_spmd(nc, in_maps, core_ids=list(range(N_CORES)), **kw)
    out = np.stack([res.results[i]["out"] for i in range(N_CORES)], axis=0)
    return out, res


def kernel(queries, keys, values, W_q, b_q, W_k, b_k, v_a, b_a=None, **_):
    # b_a shifts all scores equally -> softmax-invariant -> unused.
    nc = _get_nc()
    in_maps = make_in_maps(queries, keys, values, W_q, b_q, W_k, b_k, v_a)
    # The kernel is deterministic, but the shared device has shown rare
    # transient execution corruption: require two consecutive runs to
    # agree bit-exactly before returning.
    prev = None
    for _ in range(5):
        out, _res = run(nc, in_maps)
        if prev is not None and np.array_equal(out, prev):
            break
        prev = out
    return out.astype(np.float32)
